# revision 76
# baseline (speedup 1.0000x reference)
"""Trainium2 Bass kernel for nn_CausalPrefixAttention (8-core SPMD), v4.

Sharding: core = (batch b, head-group hg); each core projects its 128
INNER columns (2 heads) for the full sequence, runs attention for its
heads, and out-projects through its Wo rows; the 4 head-group partials
per batch are summed on the host (with bo).

vs v2 (119.6us -> 94.4us):
  - cx is never loaded in natural layout: 8 XBAR DMA-transposes load cxT
    straight from HBM into SBUF, removing 64 PE transposes and 8 big
    PSUM->SBUF copies. ALL XBAR transposes must share one queue: two
    concurrent XBAR DMAs on different queues corrupt each other (measured
    on device; per-16-token stripes of garbage).
  - x loads natural in 4 pair-DMAs (bn_stats needs tokens-on-partitions);
    PE transposes it per-tile during the DMA-bound head, copies on ACT,
    bn_stats on DVE with the scalar post-processing batched+strided.
  - DMA issue costs ~1.2us (SEQ+HWDGE) and ACT-queue issues block ACT
    engine work, so the sync queue carries x+cxT in consumption order and
    all weight/const blobs go via one scalar... (b1/b2 split so identb
    and the win q-block land early). The tile scheduler chains DMAs
    cross-queue in *consumption* order - fine-grained interleaving of
    consumers from different queues serializes the streams (6.4us/item).
  - projections are per-token-half tiles; q/kin run before stats-rows in
    a 4-tag PSUM ring, vin after; the g1-half q/kin chains stream into
    attention-g0's exp-wait gaps (PSUM: the then-idle fin tags).
  - sim PSUM is one [128,1024] f32 2-bank tile per j-tile (h0|h1), one
    strided exp per j-tile; tri-masking on gpsimd; 1/l via reciprocal
    straight from the PSUM l-row; out-projection copies on DVE so ACT
    stays pure-exp during attention (attention is exp-rate-bound).
  - finals: g0's out-projection tiles interleave into g1's attention;
    g1's j-order is cx0..6,in0..5,cx7,in6,in7 with per-span PSUM stop
    flags so token cols [0:256] finish at cx7 and only half the final
    remains after the last PV (smaller tail).
"""

import os
import sys

for _p in ("/opt/trn_rl_repo", "/root/.axon_site/_ro/trn_rl_repo"):
    if os.path.isdir(_p) and _p not in sys.path:
        sys.path.append(_p)

import numpy as np

import concourse.mybir as mybir
import concourse.tile as tile
from concourse import bacc
from concourse.bass_utils import run_bass_kernel_spmd

F32 = mybir.dt.float32
BF16 = mybir.dt.bfloat16
AF = mybir.ActivationFunctionType
ALU = mybir.AluOpType

B, N, M, DIM, INNER, HEADS, DH = 2, 1024, 1024, 1024, 512, 8, 64
EPS = 1e-5
NT = N // 128      # token tiles per batch (8)
KC = DIM // 128    # contraction chunks (8)

# blob1 column offsets (bf16): wcx | idb | tri | pick | sel (row 0, 2x128)
B1_WCX, B1_IDB, B1_TRI, B1_PICK, B1_SEL = 0, 2048, 2176, 2304, 2432
B1_COLS = 2688
# blob2: win q-block | k-block | v-block | wo (split DMA: q early, rest later)
B2_WQ, B2_WK, B2_WV, B2_WO = 0, 1152, 2304, 3456
B2_COLS = 4480


def build_program(unroll=1, phase=2):
    nc = bacc.Bacc("TRN2", target_bir_lowering=False, debug=False)

    x_d = nc.dram_tensor("x", [N, DIM], BF16, kind="ExternalInput")
    cx_d = nc.dram_tensor("cx", [M, DIM], BF16, kind="ExternalInput")
    b1_d = nc.dram_tensor("b1", [128, B1_COLS], BF16, kind="ExternalInput")
    b2_d = nc.dram_tensor("b2", [128, B2_COLS], BF16, kind="ExternalInput")
    o_d = nc.dram_tensor("o", [N, DIM], BF16, kind="ExternalOutput")

    with tile.TileContext(nc) as tc:
        for _ in range(unroll):
            _emit(nc, tc, x_d, cx_d, b1_d, b2_d, o_d, phase)
    nc.compile()
    return nc


def _emit(nc, tc, x_d, cx_d, b1_d, b2_d, o_d, phase=2):
    from contextlib import ExitStack

    ctx = ExitStack()
    with ctx:
        wpool = ctx.enter_context(tc.tile_pool(name="wpool", bufs=1))
        projp = ctx.enter_context(tc.tile_pool(name="projp", bufs=8))
        vnp = ctx.enter_context(tc.tile_pool(name="vnp", bufs=4))
        ppool = ctx.enter_context(tc.tile_pool(name="ppool", bufs=10))
        otp = ctx.enter_context(tc.tile_pool(name="otp", bufs=4))
        ostp = ctx.enter_context(tc.tile_pool(name="ostp", bufs=6))
        tiny = ctx.enter_context(tc.tile_pool(name="tiny", bufs=8))
        consts = ctx.enter_context(tc.tile_pool(name="consts", bufs=1))

        eps_col = consts.tile([128, 1], F32)
        nc.vector.memset(eps_col, EPS)
        ones_col2 = consts.tile([128, 8], BF16)
        nc.vector.memset(ones_col2, 1.0)

        # ---- input DMA stream. DMA issue costs ~1.2us each on the HWDGE
        # queues (SEQ+HWDGE) and ACT-queue issues block ACT engine work, so:
        # sync queue = x pair-loads + the 8 XBAR transposes (consumption
        # order); Pool/SWDGE queue = all weight/const blobs (desc-gen runs
        # on the idle Pool engine, 25ns SEQ). Scalar issues nothing early.
        b1 = wpool.tile([128, B1_COLS], BF16, tag="b1")
        b2 = wpool.tile([128, B2_COLS], BF16, tag="b2")
        natx = ctx.enter_context(tc.tile_pool(name="natx", bufs=1))
        xnat_t = natx.tile([128, NT, DIM], BF16, tag="nat", name="xnat")
        x_r = x_d.rearrange("(t p) d -> p t d", p=128)
        for hp in range(NT // 2):
            nc.sync.dma_start(out=xnat_t[:, 2 * hp:2 * hp + 2, :],
                              in_=x_r[:, 2 * hp:2 * hp + 2, :])
        x_nat = [xnat_t[:, t, :] for t in range(NT)]

        # cxT via XBAR DMA transpose, chunk-major (single queue — see above)
        cxT_t = wpool.tile([128, KC, M], BF16, tag="cxT")
        for c in range(KC):
            nc.sync.dma_start(out=cxT_t[:, c, :],
                              in_=cx_d[:, c * 128:(c + 1) * 128],
                              transpose=True)

        nc.gpsimd.dma_start(out=b1[:, B1_IDB:B1_IDB + 128],
                            in_=b1_d[:, B1_IDB:B1_IDB + 128])
        nc.gpsimd.dma_start(out=b1[:, B1_IDB + 128:], in_=b1_d[:, B1_IDB + 128:])
        nc.gpsimd.dma_start(out=b2[:, 0:B2_WK], in_=b2_d[:, 0:B2_WK])
        nc.gpsimd.dma_start(out=b2[:, B2_WK:B2_WV], in_=b2_d[:, B2_WK:B2_WV])
        nc.gpsimd.dma_start(out=b1[:, 0:B1_IDB], in_=b1_d[:, 0:B1_IDB])
        nc.gpsimd.dma_start(out=b2[:, B2_WV:], in_=b2_d[:, B2_WV:])
        wcx = b1[:, B1_WCX:B1_WCX + 2048].rearrange("p (c k) -> p c k", k=256)
        identb = b1[:, B1_IDB:B1_IDB + 128]
        tri = b1[:, B1_TRI:B1_TRI + 128]
        pick3 = b1[0:3, B1_PICK:B1_PICK + 128]
        sel2 = b1[0:1, B1_SEL:B1_SEL + 256]
        winq = b2[:, B2_WQ:B2_WQ + 1152].rearrange("p (c k) -> p c k", k=128)
        wink = b2[:, B2_WK:B2_WK + 1152].rearrange("p (c k) -> p c k", k=128)
        winv = b2[:, B2_WV:B2_WV + 1152].rearrange("p (c k) -> p c k", k=128)
        wo = b2[:, B2_WO:B2_WO + 1024]
        # stat rows: row0 = -mu, row1 = std (aug contraction), row2 = rs
        srow = consts.tile([3, N], BF16)

        # per-token-half projection tiles: no false write-after-read deps
        # when the g1-half chains stream into attention-g0
        kcxT = projp.tile([128, M], BF16, tag="proj", name="kcxT")
        vcxT = projp.tile([128, M], BF16, tag="proj", name="vcxT")
        qTg = [projp.tile([128, 512], BF16, tag="proj", name=f"qT{g}")
               for g in range(2)]
        kinTg = [projp.tile([128, 512], BF16, tag="proj", name=f"kinT{g}")
                 for g in range(2)]
        vinTg = [projp.tile([128, 512], BF16, tag="proj", name=f"vinT{g}")
                 for g in range(2)]
        rsb = ctx.enter_context(tc.tile_pool(name="rsb", bufs=2))
        rs_bc = [rsb.tile([128, 512], F32, tag="rsbc", name=f"rsbc{g}")
                 for g in range(2)]
        vn = [None] * 16

        phase_a = ExitStack()
        with phase_a:
            tposed = phase_a.enter_context(tc.tile_pool(name="tposed", bufs=1))
            psA = phase_a.enter_context(
                tc.tile_pool(name="psA", bufs=1, space="PSUM"))

            # ---- x transposes on PE, one x-tile per PSUM batch so the
            # first batch only needs x tile 0; copies on ACT; bn_stats on
            # DVE per tile, post-processing batched at the end ----
            xT = tposed.tile([128, 2, KC, 512], BF16, tag="tp", name="xT")
            s4a = tiny.tile([128, NT, 4], F32, tag="s4a", name="s4a")
            for t in range(NT):
                ps = psA.tile([128, 1024], BF16, tag="tps", bufs=2,
                              name="tps")
                for c in range(KC):
                    nc.tensor.transpose(
                        ps[:, c * 128:(c + 1) * 128],
                        x_nat[t][:, c * 128:(c + 1) * 128], identb)
                co = (t % 4) * 128
                nc.scalar.copy(
                    out=xT[:, t // 4, :, co:co + 128],
                    in_=ps.rearrange("p (c k) -> p c k", k=128))
                bst = tiny.tile([128, 2, 6], F32, tag="bst", name="bst")
                for half in range(2):
                    nc.vector.bn_stats(
                        out=bst[:, half, :],
                        in_=x_nat[t][:, half * 512:(half + 1) * 512])
                nc.vector.bn_aggr(out=s4a[:, t, 0:2], in_=bst)

            # batched stats post-processing: cols 0:2 = (mu, var) per tile;
            # -> col 0 = -mu, col 1 = std, col 2 = rs
            nc.scalar.activation(
                out=s4a.rearrange("p t k -> p (t k)")[:, 1::4],
                in_=s4a.rearrange("p t k -> p (t k)")[:, 1::4],
                func=AF.Sqrt, bias=eps_col)
            nc.vector.reciprocal(
                out=s4a.rearrange("p t k -> p (t k)")[:, 2::4],
                in_=s4a.rearrange("p t k -> p (t k)")[:, 1::4])
            nc.vector.tensor_scalar(
                out=s4a.rearrange("p t k -> p (t k)")[:, 0::4],
                in0=s4a.rearrange("p t k -> p (t k)")[:, 0::4],
                scalar1=-1.0, scalar2=None, op0=ALU.mult)
            s4b = tiny.tile([128, NT, 3], BF16, tag="s4b", name="s4b")
            nc.vector.tensor_copy(out=s4b, in_=s4a[:, :, 0:3])

            def in_chain(w9, dst, gg, pool, tag, bufs):
                """One input-projection half: 8 chunks + aug, rs on copy-out.
                Returns the matmul/copy thunks for interleaved emission."""
                st = {}
                sp = slice(gg * 512, (gg + 1) * 512)

                def step(c):
                    if c == 0:
                        st["ps"] = pool.tile([128, 512], F32, tag=tag,
                                             bufs=bufs, name=tag)
                    nc.tensor.matmul(
                        st["ps"], w9[:, c, :], xT[:, gg, c, :],
                        start=(c == 0), stop=False)

                def aug():
                    nc.tensor.matmul(
                        st["ps"], w9[0:2, KC, :], srow[0:2, sp],
                        start=False, stop=True)
                    nc.vector.tensor_tensor(
                        out=dst, in0=st["ps"], in1=rs_bc[gg], op=ALU.mult)

                return [lambda c=c: step(c) for c in range(KC)] + [aug]

            # projections run in pairs around the stats-row build so the
            # 4-tag PSUM ring always has 2 free banks for rs_bc/next pair
            pair_a = [in_chain(winq, qTg[0], 0, psA, "pps0", 1),
                      in_chain(wink, kinTg[0], 0, psA, "pps1", 1)]
            for ch in pair_a:
                for th in ch[:-1]:
                    th()

            # ---- stats rows (PE transposes are tiny; stats long done) ----
            for t in range(NT):
                ps2 = psA.tile([128, 512], BF16, tag="tpsr", bufs=2,
                               name="tpsr")
                nc.tensor.transpose(ps2[0:3, 0:128], s4b[:, t, :], identb)
                nc.vector.tensor_copy(
                    out=srow[:, t * 128:(t + 1) * 128], in_=ps2[0:3, 0:128])
            # rs broadcast tiles: pick3^T selects srow row 2 into every part
            for g in range(2):
                ps = psA.tile([128, 512], F32, tag=f"pps{2 + g}", bufs=1,
                              name=f"pps{2 + g}")
                nc.tensor.matmul(
                    ps, pick3, srow[:, g * 512:(g + 1) * 512],
                    start=True, stop=True)
                nc.scalar.copy(out=rs_bc[g], in_=ps)
            for ch in pair_a:
                ch[-1]()

            pair_b = [in_chain(winv, vinTg[0], 0, psA, "pps0", 1),
                      in_chain(winv, vinTg[1], 1, psA, "pps1", 1)]
            for ch in pair_b:
                for th in ch:
                    th()

            # ---- context projections (cxT streamed by the DMA queue);
            # copies on ACT ----
            ci = 0
            for pj, dst in ((0, kcxT), (1, vcxT)):
                for gg in (0, 1):
                    sp = slice(gg * 512, (gg + 1) * 512)
                    ps = psA.tile([128, 512], F32, tag=f"pps{(2 + ci) % 4}",
                                  bufs=1, name=f"pps{(2 + ci) % 4}")
                    ci += 1
                    for c in range(KC):
                        nc.tensor.matmul(
                            ps, wcx[:, c, pj * 128:(pj + 1) * 128],
                            cxT_t[:, c, sp],
                            start=(c == 0), stop=(c == KC - 1))
                    nc.scalar.copy(out=dst[:, sp], in_=ps)

            # v_nat tiles: 4 j's per [128, 520] tile, each j = [64 vfeat h0 |
            # ones | 64 vfeat h1 | ones] so the PV stationary is contiguous.
            def v_transpose_half(src512, base):
                v_t = vnp.tile([128, 520], BF16, tag="vn", name=f"vn{base}")
                for jj in range(4):
                    vn[base + jj] = (v_t, jj)
                ps = psA.tile([128, 512], BF16, tag="tpsr", bufs=2,
                              name="tpsr")
                for jj in range(4):
                    nc.tensor.transpose(
                        ps[:, jj * 128:(jj + 1) * 128],
                        src512[:, jj * 128:(jj + 1) * 128], identb)
                nc.gpsimd.tensor_copy(
                    out=v_t.rearrange("p (a b) -> p a b", b=65)[:, :, 64:65],
                    in_=ones_col2.rearrange("p (a b) -> p a b", b=1))
                nc.vector.tensor_copy(
                    out=v_t.rearrange("p (a b) -> p a b", b=65)[:, :, 0:64],
                    in_=ps.rearrange("p (a b) -> p a b", b=64))

            v_transpose_half(vinTg[0], 8)
            v_transpose_half(vinTg[1], 12)
            v_transpose_half(vcxT[:, 0:512], 0)
            v_transpose_half(vcxT[:, 512:1024], 4)

            # dummy exp: forces the Exp act-table load off the attention
            # start (the load costs ~1.3us on ACT)
            junk = tiny.tile([128, 1], BF16, tag="junk", name="junk")
            nc.scalar.activation(out=junk, in_=eps_col, func=AF.Exp)

            if phase == 1:
                for t, src_t in enumerate((qTg[0], kinTg[0], vinTg[0],
                                           kcxT[:, 0:512], vcxT[:, 0:512],
                                           qTg[1], kinTg[1], vinTg[1])):
                    nc.sync.dma_start(
                        out=o_d[t * 128:(t + 1) * 128, 0:512].bitcast(BF16),
                        in_=src_t)
                return

        # ---- attention + final projection ----
        with tc.tile_pool(name="psSim", bufs=1, space="PSUM") as psS, \
             tc.tile_pool(name="psO", bufs=1, space="PSUM") as psO, \
             tc.tile_pool(name="psF", bufs=1, space="PSUM") as psF:
            pend_final = [None]

            def final_head(g, o_ps):
                """lrec/lbc/oT chain. MUST be fully emitted before the next
                g's first PV (o_ps ring reuse is ordered by emission)."""
                lrec = [tiny.tile([1, 512], BF16, tag=f"lr{h}", bufs=3,
                                  name=f"lr{h}") for h in (0, 1)]
                with nc.allow_low_precision(reason="1/l in bf16 is plenty"):
                    for h in (0, 1):
                        nc.vector.reciprocal(out=lrec[h],
                                             in_=o_ps[h][64:65, :])
                lbc_ps = psF.tile([128, 512], F32, tag="fin0", bufs=1,
                                  name="lbc")
                for h in (0, 1):
                    nc.tensor.matmul(
                        lbc_ps, sel2[:, 128 * h:128 * h + 128], lrec[h],
                        start=(h == 0), stop=(h == 1))
                lbc = tiny.tile([128, 512], F32, tag="lbc", bufs=3,
                                name="lbc")
                nc.vector.tensor_copy(out=lbc, in_=lbc_ps)
                oT = otp.tile([128, 512], BF16, tag="oT")
                for h in (0, 1):
                    nc.vector.tensor_tensor(
                        out=oT[64 * h:64 * h + 64, :], in0=o_ps[h][0:64, :],
                        in1=lbc[64 * h:64 * h + 64, :], op=ALU.mult)
                pend_final[0] = None
                return oT

            def fin_tile(g, oT, t, tail):
                """Out-projection + store for one 128-token tile."""
                o_r = o_d.rearrange("(t p) d -> p t d", p=128)
                ost = ostp.tile([128, 1, DIM], BF16, tag="ost")
                for half in range(2):
                    wsp = slice(half * 512, (half + 1) * 512)
                    fp = psF.tile([128, 512], F32, tag=f"fin{half}",
                                  bufs=1, name=f"fin{half}")
                    nc.tensor.matmul(
                        fp, oT[:, t * 128:(t + 1) * 128], wo[:, wsp],
                        start=True, stop=True)
                    # at the tail ACT is idle (no more exp) — alternate
                    if tail and half == 1:
                        nc.scalar.copy(out=ost[:, 0, wsp], in_=fp)
                    else:
                        nc.vector.tensor_copy(out=ost[:, 0, wsp], in_=fp)
                nc.sync.dma_start(out=o_r[:, g * 4 + t:g * 4 + t + 1, :],
                                  in_=ost)

            # overlay work: q/kin g1 chains (PSUM: the idle fin tags) stream
            # into attention-g0's exp-wait gaps; final-g0's out-projection
            # tiles stream into attention-g1's.
            overlay = (in_chain(winq, qTg[1], 1, psF, "fin0", 1)
                       + in_chain(wink, kinTg[1], 1, psF, "fin1", 1))

            def final_half(o_ps, ca, cb, tiles, tail):
                """Normalize + out-project token cols [ca:cb] (g=1 halves)."""
                w = cb - ca
                lrec = [tiny.tile([1, 512], BF16, tag=f"lr{h}", bufs=3,
                                  name=f"lr{h}") for h in (0, 1)]
                with nc.allow_low_precision(reason="1/l in bf16 is plenty"):
                    for h in (0, 1):
                        nc.vector.reciprocal(out=lrec[h][:, 0:w],
                                             in_=o_ps[h][64:65, ca:cb])
                lbc_ps = psF.tile([128, 512], F32, tag="fin0", bufs=1,
                                  name="lbc")
                for h in (0, 1):
                    nc.tensor.matmul(
                        lbc_ps[:, ca:cb], sel2[:, 128 * h:128 * h + 128],
                        lrec[h][:, 0:w], start=(h == 0), stop=(h == 1))
                lbc = tiny.tile([128, 512], F32, tag="lbc", bufs=3,
                                name="lbc")
                nc.vector.tensor_copy(out=lbc[:, ca:cb], in_=lbc_ps[:, ca:cb])
                oT = otp.tile([128, 512], BF16, tag="oT")
                for h in (0, 1):
                    nc.vector.tensor_tensor(
                        out=oT[64 * h:64 * h + 64, ca:cb],
                        in0=o_ps[h][0:64, ca:cb],
                        in1=lbc[64 * h:64 * h + 64, ca:cb], op=ALU.mult)
                for t in tiles:
                    fin_tile(1, oT, t, tail=tail)

            prev_g = [None]
            for g in (0, 1):
                # g0: cx0..6, in0..3, cx7 (stop on the full final span).
                # g1: cx0..6, in0..5, cx7, in6, in7 — token cols [0:256] are
                # fully accumulated at cx7, so the out-projection for token
                # tiles 0,1 runs while in6/in7 still stream (smaller tail).
                j_list = [("cx", j) for j in range(7)]
                if g == 0:
                    j_list += [("in", j) for j in range(4)]
                    j_list.append(("cx", 7))
                else:
                    j_list += [("in", j) for j in range(6)]
                    j_list += [("cx", 7), ("in", 6), ("in", 7)]
                n_j = len(j_list)
                o_ps = [psO.tile([128, 512], F32, tag=f"o{h}", name=f"ops{h}")
                        for h in (0, 1)]

                def j_meta(idx, g=g, j_list=j_list):
                    src, j = j_list[idx]
                    if src == "cx":
                        return kcxT, j, j, 0, False
                    off = max(0, 128 * (j - 4 * g))
                    return None, j, 8 + j, off, j >= 4 * g

                sims = [None] * n_j

                def emit_sim(idx, j_meta=j_meta, sims=sims, g=g):
                    kT, j, jg, off, diag = j_meta(idx)
                    ps = psS.tile([128, 1024], F32, tag="sim", bufs=2,
                                  name="sim")
                    for h in (0, 1):
                        hsl = slice(64 * h, 64 * h + 64)
                        if kT is None:
                            kop = kinTg[j // 4][hsl, (j % 4) * 128:
                                                (j % 4) * 128 + 128]
                        else:
                            kop = kT[hsl, j * 128:(j + 1) * 128]
                        nc.tensor.matmul(
                            ps[:, 512 * h + off:512 * (h + 1)],
                            kop, qTg[g][hsl, off:512],
                            start=True, stop=True)
                    sims[idx] = ps

                # software pipeline: sim for j+1 is emitted before PV of j so
                # the in-order PE computes the next sim while ACT runs exp.
                emit_sim(0)
                fin_steps = []
                if pend_final[0] is not None:
                    oT_prev = final_head(prev_g[0], pend_final[0])
                    fin_steps = [(prev_g[0], oT_prev, t) for t in range(4)]
                for idx in range(n_j):
                    if idx + 1 < n_j:
                        emit_sim(idx + 1)
                    if fin_steps and idx >= 4 and idx % 3 == 1:
                        pg, oTp, t = fin_steps.pop(0)
                        fin_tile(pg, oTp, t, tail=False)
                    for _ in range(2):
                        if overlay:
                            overlay.pop(0)()
                    kT, j, jg, off, diag = j_meta(idx)
                    p_t = ppool.tile([128, 1024], BF16, tag="p", name="p")
                    ps3 = sims[idx].rearrange("p (h t) -> p h t", h=2)
                    p3 = p_t.rearrange("p (h t) -> p h t", h=2)
                    nc.scalar.activation(
                        out=p3[:, :, off:512], in_=ps3[:, :, off:512],
                        func=AF.Exp)
                    if diag:
                        for h in (0, 1):
                            nc.gpsimd.tensor_tensor(
                                out=p_t[:, 512 * h + off:512 * h + off + 128],
                                in0=p_t[:, 512 * h + off:512 * h + off + 128],
                                in1=tri, op=ALU.mult)
                    sims[idx] = None
                    v_t, jj = vn[jg]
                    if g == 0:
                        spans = [(off, 512, idx == n_j - 1)]
                    elif idx == 13:    # cx7: [0:256] complete
                        spans = [(0, 256, True), (256, 512, False)]
                    elif idx == 14:    # in6 (off 256): [256:384] complete
                        spans = [(256, 384, True), (384, 512, False)]
                    elif idx == 15:    # in7 (off 384)
                        spans = [(384, 512, True)]
                    else:
                        spans = [(off, 512, False)]
                    for lo, hi, stop in spans:
                        for h in (0, 1):
                            nc.tensor.matmul(
                                o_ps[h][0:65, lo:hi],
                                v_t[:, 130 * jj + 65 * h:
                                    130 * jj + 65 * h + 65],
                                p_t[:, 512 * h + lo:512 * h + hi],
                                start=(idx == 0), stop=stop)
                    if g == 1 and idx == 13:
                        final_half(o_ps, 0, 256, (0, 1), tail=False)
                for th in overlay:
                    th()
                overlay = []
                for pg, oTp, t in fin_steps:
                    fin_tile(pg, oTp, t, tail=False)
                pend_final[0] = o_ps
                prev_g[0] = g
            final_half(pend_final[0], 256, 512, (2, 3), tail=True)


_NC_CACHE = None


def _get_nc():
    global _NC_CACHE
    if _NC_CACHE is None:
        _NC_CACHE = build_program()
    return _NC_CACHE


def make_in_maps(x, context, gamma, beta, Wq, Wkv, Wo, bo):
    import ml_dtypes
    BF = ml_dtypes.bfloat16
    x = np.asarray(x, np.float32)
    context = np.asarray(context, np.float32)
    gamma = np.asarray(gamma, np.float32)
    beta = np.asarray(beta, np.float32)
    Wq = np.asarray(Wq, np.float32)
    Wkv = np.asarray(Wkv, np.float32)
    Wo = np.asarray(Wo, np.float32)

    s = DH ** -0.5
    in_maps = []
    for core in range(8):
        b, hg = divmod(core, 4)
        cols = slice(128 * hg, 128 * hg + 128)
        wq = Wq[:, cols] * gamma[:, None] * s
        uq = wq.sum(0)
        bq = beta @ Wq[:, cols] * s
        wk = Wkv[:, :INNER][:, cols] * gamma[:, None]
        uk = wk.sum(0)
        bk = beta @ Wkv[:, :INNER][:, cols]
        wv = Wkv[:, INNER:][:, cols] * gamma[:, None]
        uv = wv.sum(0)
        bv = beta @ Wkv[:, INNER:][:, cols]

        # per-projection 9-chunk blocks (chunk 8 = aug rows u, b)
        def blk(w, u, bvec):
            out = np.zeros((128, KC + 1, 128), np.float32)
            for c in range(KC):
                out[:, c, :] = w[128 * c:128 * c + 128]
            out[0, KC, :] = u
            out[1, KC, :] = bvec
            return out.reshape(128, 1152)

        wcx = np.zeros((128, KC, 256), np.float32)
        for c in range(KC):
            rows = slice(128 * c, 128 * c + 128)
            wcx[:, c, 0:128] = Wkv[:, :INNER][rows, cols]
            wcx[:, c, 128:256] = Wkv[:, INNER:][rows, cols]

        b1 = np.zeros((128, B1_COLS), np.float32)
        b1[:, B1_WCX:B1_WCX + 2048] = wcx.reshape(128, 2048)
        b1[:, B1_IDB:B1_IDB + 128] = np.eye(128, dtype=np.float32)
        b1[:, B1_TRI:B1_TRI + 128] = np.tril(np.ones((128, 128), np.float32)).T
        b1[2, B1_PICK:B1_PICK + 128] = 1.0
        b1[0, B1_SEL:B1_SEL + 64] = 1.0
        b1[0, B1_SEL + 192:B1_SEL + 256] = 1.0

        b2 = np.zeros((128, B2_COLS), np.float32)
        b2[:, B2_WQ:B2_WQ + 1152] = blk(wq, uq, bq)
        b2[:, B2_WK:B2_WK + 1152] = blk(wk, uk, bk)
        b2[:, B2_WV:B2_WV + 1152] = blk(wv, uv, bv)
        b2[:, B2_WO:B2_WO + 1024] = Wo[cols, :]

        in_maps.append({
            "x": np.ascontiguousarray(x[b]).astype(BF),
            "cx": np.ascontiguousarray(context[b]).astype(BF),
            "b1": b1.astype(BF),
            "b2": b2.astype(BF),
        })
    return in_maps


def assemble(results, bo):
    bo = np.asarray(bo, np.float32)
    out = np.zeros((B, N, DIM), np.float32)
    for core in range(8):
        b = core // 4
        out[b] += results[core]["o"].astype(np.float32)
    out += bo[None, None, :]
    return out


def kernel(x, context, gamma, beta, Wq, Wkv, Wo, bo):
    nc = _get_nc()
    in_maps = make_in_maps(x, context, gamma, beta, Wq, Wkv, Wo, bo)
    res = run_bass_kernel_spmd(nc, in_maps, list(range(8)))
    return assemble(res.results, bo)
